# revision 71
# baseline (speedup 1.0000x reference)
"""GQA causal attention (llama3-style RoPE) on 8 TRN2 NeuronCores.

Sharding: tensor-parallel over heads. Core c gets q-heads 4c..4c+3 and
kv-head c (GQA groups intact), plus the matching row-block of wo.T.
Each core computes a full [S, D] partial of the output projection;
the host sums the 8 partials (the "all-reduce" of the row-sharded wo).

Per-core pipeline (stages P=proj+rope, S=scores+exp, V=av+norm, W=wo
emitted software-pipelined: P0 P1 S0 P2 V0 S1 P3 V1 S2 V2 S3a W0 S3b W1
V3a W2 (V3b_r+W3_sm interleaved) so PE streams while ACT runs exp):
  qkvT[col, js] = wqkvT.T @ xT    fp8e4m3 hi/lo 3-term DoubleRow
                                  (weights host-scaled x256; descale
                                  via exp scale and the v drain)
  RoPE on qT/kT                   two q heads stacked per [128, 512]
  v -> vaug[sk, hd|1]             PE transpose + ones column
  sT[sk, 2, sq] = kT.T @ qT       f16, K=64 row-groups: head-even rows
                                  0:64, head-odd rows 64:128 (kt dup)
  eT = exp(sT/(8*WS^2) - 3)       one ACT op per [128, 2, 512-off];
                                  the -3 shift cancels in the softmax
  av[sq, 2*(hd|den)] = et.T @ vaug   et slabs stationary (LdWeights is
                                  free), 65-wide moving operand, ones
                                  column gives the denominator
  y2 = av * recip(den)            DVE reciprocal + tensor_scalar mult
  yt8 = fp8_hi/lo(transpose(y2))  SP xbar dma-transpose, Pool converts
  out[sq, d] = yt8.T @ woT8       fp8 hi/lo 3-term DoubleRow (wot
                                  x256); bf16 partials, host sums all
                                  cores and divides by 256.
GPSIMD never touches PSUM (hw constraint); all psum drains are DVE/ACT.
"""

import sys

for _p in ("/opt/trn_rl_repo", "/root/.axon_site/_ro/trn_rl_repo"):
    if _p not in sys.path:
        sys.path.insert(0, _p)

import numpy as np
import ml_dtypes

import concourse.bacc as bacc
import concourse.mybir as mybir
import concourse.tile as tile

BF16 = ml_dtypes.bfloat16
F8 = ml_dtypes.float8_e4m3

S = 2048
D = 2048
HD = 64
NH = 32
NKV = 8
NCORES = 8
QH = NH // NCORES            # 4 local q heads
QCOLS = QH * HD              # 256
P = 128
NK = D // P                  # 16 contraction tiles
NKP = NK // 2                # 8 DoubleRow pairs
NSQ = S // P                 # 16 seq tiles of 128
NCH = 4                      # seq chunks of 512
CH = 512
WS = 256.0                   # fp8 weight pre-scale
CSH = 3.0                    # exp shift (cancels in softmax ratio)

_CACHE = {}


def _build():
    bf = mybir.dt.bfloat16
    f16 = mybir.dt.float16
    f32 = mybir.dt.float32
    f8 = mybir.dt.float8e4
    DR = mybir.MatmulPerfMode.DoubleRow
    Exp = mybir.ActivationFunctionType.Exp

    nc = bacc.Bacc()
    xhi_d = nc.dram_tensor("xhi", [NKP * P, 2 * S], f8, kind="ExternalInput")
    xlo_d = nc.dram_tensor("xlo", [NKP * P, 2 * S], f8, kind="ExternalInput")
    whi_d = nc.dram_tensor("whi", [NKP * P, 2 * 384], f8, kind="ExternalInput")
    wlo_d = nc.dram_tensor("wlo", [NKP * P, 2 * 384], f8, kind="ExternalInput")
    wothi_d = nc.dram_tensor("wothi", [P, 2 * D], f8, kind="ExternalInput")
    wotlo_d = nc.dram_tensor("wotlo", [P, 2 * D], f8, kind="ExternalInput")
    cos_d = nc.dram_tensor("cos128", [P, S], f16, kind="ExternalInput")
    swap_d = nc.dram_tensor("swap128", [P, S], f16, kind="ExternalInput")
    masks_d = nc.dram_tensor("masks2", [P, 2 * P], bf, kind="ExternalInput")
    ident_d = nc.dram_tensor("ident", [HD, HD], bf, kind="ExternalInput")
    out_d = nc.dram_tensor("out", [S, D], bf, kind="ExternalOutput")

    with tile.TileContext(nc) as tc:
        with (
            tc.tile_pool(name="const", bufs=1) as cpool,
            tc.tile_pool(name="xw", bufs=16) as xwpool,
            tc.tile_pool(name="vaug", bufs=16) as vpool,
            tc.tile_pool(name="et", bufs=33) as epool,
            tc.tile_pool(name="rope", bufs=2) as rpool,
            tc.tile_pool(name="y2", bufs=4) as ypool,
            tc.tile_pool(name="ot", bufs=3) as opool,
            tc.tile_pool(name="ps_a", bufs=2, space="PSUM") as ps_a,
            tc.tile_pool(name="ps_s", bufs=2, space="PSUM") as ps_s,
            tc.tile_pool(name="ps_av", bufs=2, space="PSUM") as ps_av,
        ):
            # ---- constants + weights + x in ------------------------------
            cos_sb = cpool.tile([P, S], f16, tag="cos")
            swap_sb = cpool.tile([P, S], f16, tag="swap")
            masks_sb = cpool.tile([P, 2, P], bf, tag="masks")
            ident_sb = cpool.tile([HD, HD], bf, tag="ident")
            cbias = cpool.tile([P, 1], f32, tag="cbias")
            nc.gpsimd.memset(cbias[:], -CSH)
            nc.gpsimd.dma_start(masks_sb[:], masks_d[:])
            nc.gpsimd.dma_start(ident_sb[:], ident_d[:])

            whi_sb, wlo_sb = [], []
            for kp in range(NKP):
                for lst, dram, nm in ((whi_sb, whi_d, "whi"), (wlo_sb, wlo_d, "wlo")):
                    w = xwpool.tile([P, 2, 384], f8, tag="w", name=f"{nm}{kp}")
                    eng = [nc.sync, nc.scalar][kp % 2]
                    eng.dma_start(w[:], dram[kp * P : (kp + 1) * P, :])
                    lst.append(w)

            xhi_sb, xlo_sb = [], []
            for kp in range(NKP):
                xhi_sb.append(xwpool.tile([P, 2, S], f8, tag="x", name=f"xhi{kp}"))
                xlo_sb.append(xwpool.tile([P, 2, S], f8, tag="x", name=f"xlo{kp}"))
            # x loads in 3 col-groups: [0:512), [512:1024), [1024:2048)
            for gi, (g0, g1) in enumerate(((0, CH), (CH, 2 * CH), (2 * CH, S))):
                for kp in range(NKP):
                    for xi, (sb_t, dram) in enumerate(
                        ((xhi_sb, xhi_d), (xlo_sb, xlo_d))
                    ):
                        src3 = dram[kp * P : (kp + 1) * P, :].rearrange(
                            "p (two s) -> p two s", two=2
                        )
                        eng = (
                            [nc.sync, nc.scalar][(kp + xi) % 2]
                            if gi < 2
                            else [nc.sync, nc.gpsimd][(kp + xi) % 2]
                        )
                        eng.dma_start(sb_t[kp][:, :, g0:g1], src3[:, :, g0:g1])

            nc.scalar.dma_start(cos_sb[:], cos_d[:])
            nc.scalar.dma_start(swap_sb[:], swap_d[:])
            wothi_sb = cpool.tile([P, 2, D], f8, tag="wothi")
            wotlo_sb = cpool.tile([P, 2, D], f8, tag="wotlo")
            nc.scalar.dma_start(wothi_sb[:], wothi_d[:])
            nc.scalar.dma_start(wotlo_sb[:], wotlo_d[:])

            qt_sb = [cpool.tile([P, S], f16, tag=f"qt{m}", name=f"qt{m}") for m in range(2)]
            kt_sb = cpool.tile([P, S], f16, tag="kt")
            vt_ch = [None] * NCH
            yt8hi_sb = cpool.tile([P, 2, S], f8, tag="yt8hi")
            yt8lo_sb = cpool.tile([P, 2, S], f8, tag="yt8lo")
            vaug_sb = [None] * NSQ

            # ---- helpers -------------------------------------------------
            def proj_psum(m, j):
                js = slice(j * CH, (j + 1) * CH)
                ps = ps_a.tile([P, CH], f32, tag="proj", name="ps_proj")
                with nc.named_scope("proj"):
                    n = 0
                    for kp in range(NKP):
                        for wt, xt in (
                            (whi_sb[kp], xhi_sb[kp]),
                            (whi_sb[kp], xlo_sb[kp]),
                            (wlo_sb[kp], xhi_sb[kp]),
                        ):
                            nc.tensor.matmul(
                                ps[:],
                                wt[:, :, m * P : (m + 1) * P],
                                xt[:, :, js],
                                start=(n == 0),
                                stop=(n == 3 * NKP - 1),
                                perf_mode=DR,
                            )
                            n += 1
                return ps

            def rope128(dst, ps, j):
                # two heads stacked: rows 0:64 head-even, 64:128 head-odd
                js = slice(j * CH, (j + 1) * CH)
                with nc.named_scope("rope"):
                    qr = rpool.tile([P, CH], f16, tag="qr", name="qr")
                    nc.vector.tensor_copy(qr[:], ps[:])
                    t2 = rpool.tile([P, CH], f16, tag="t2", name="t2")
                    for b in range(4):
                        d0 = b * 32
                        s0 = (b ^ 1) * 32
                        eng = nc.vector if b < 2 else nc.gpsimd
                        eng.tensor_mul(
                            t2[d0 : d0 + 32, :], qr[s0 : s0 + 32, :], swap_sb[s0 : s0 + 32, js]
                        )
                    nc.vector.tensor_mul(dst[:, js], qr[:], cos_sb[:, js])
                    nc.vector.tensor_add(dst[:, js], dst[:, js], t2[:])

            def rope_kv(ps, j):
                js = slice(j * CH, (j + 1) * CH)
                with nc.named_scope("rope"):
                    qr = rpool.tile([P, CH], f16, tag="qr", name="qr_k")
                    nc.vector.tensor_copy(qr[0:HD, :], ps[0:HD, :])
                    # v drain first: frees the proj psum before the rope muls
                    vtc = rpool.tile([HD, CH], bf, tag="vtc", name="vtc", bufs=2)
                    nc.vector.tensor_scalar_mul(vtc[:], ps[HD:P, :], 1.0 / WS)
                    vt_ch[j] = vtc
                    t2 = rpool.tile([P, CH], f16, tag="t2", name="t2_k")
                    for b in range(2):
                        d0 = b * 32
                        s0 = (b ^ 1) * 32
                        eng = nc.vector if b < 2 else nc.gpsimd
                        eng.tensor_mul(
                            t2[d0 : d0 + 32, :], qr[s0 : s0 + 32, :], swap_sb[s0 : s0 + 32, js]
                        )
                    nc.vector.tensor_mul(kt_sb[0:HD, js], qr[0:HD, :], cos_sb[0:HD, js])
                    nc.vector.tensor_add(kt_sb[0:HD, js], kt_sb[0:HD, js], t2[0:HD, :])
                # duplicate k rows for the odd-head row group
                nc.gpsimd.dma_start(kt_sb[HD:P, js], kt_sb[0:HD, js])

            def vtrans(j):
                with nc.named_scope("vtrans"):
                    for i in range(4 * j, 4 * j + 4):
                        pt = ps_av.tile([P, HD], bf, tag="av", name="ps_vt")
                        lo = (i - 4 * j) * P
                        nc.tensor.transpose(pt[:], vt_ch[j][:, lo : lo + P], ident_sb[:])
                        va = vpool.tile([P, HD + 1], bf, tag="vaug", name=f"vaug{i}")
                        nc.vector.tensor_copy(va[:, 0:HD], pt[:])
                        nc.gpsimd.memset(va[:, HD : HD + 1], 1.0)
                        vaug_sb[i] = va

            all_ets = {}

            def scores_chunk(j, m, ilist=None):
                nlive = 4 * j + 4
                if ilist is None:
                    ilist = range(nlive)
                    all_ets[(j, m)] = []
                ets = all_ets.setdefault((j, m), [])
                with nc.named_scope("scores"):
                    for i in ilist:
                        off = max(0, i - 4 * j) * P
                        ps2 = ps_s.tile([P, 2, CH], f32, tag="sc", name="ps_sc")
                        for u in range(2):
                            rg = slice(u * HD, (u + 1) * HD)
                            nc.tensor.matmul(
                                ps2[:, u, off:],
                                kt_sb[rg, i * P : (i + 1) * P],
                                qt_sb[m][rg, j * CH + off : (j + 1) * CH],
                                start=True,
                                stop=True,
                            )
                        et = epool.tile([P, 2, CH], bf, tag="et", name="et")
                        with nc.named_scope("exp"):
                            nc.scalar.activation(
                                et[:, :, off:],
                                ps2[:, :, off:],
                                Exp,
                                bias=cbias[:],
                                scale=0.125 / (WS * WS),
                            )
                        if i >= 4 * j:
                            with nc.named_scope("mask"):
                                nc.gpsimd.tensor_mul(
                                    et[:, :, off : off + P],
                                    et[:, :, off : off + P],
                                    masks_sb[:],
                                )
                        ets.append(et)

            def av_chunk(j, m, rlist=(0, 1, 2, 3)):
                ets = all_ets[(j, m)]
                for r in rlist:
                    t = 4 * j + r
                    pav = ps_av.tile([P, 2 * (HD + 1)], f32, tag="av", name="ps_av")
                    with nc.named_scope("av"):
                        for u in range(2):
                            for i in range(t + 1):
                                nc.tensor.matmul(
                                    pav[:, u * (HD + 1) : (u + 1) * (HD + 1)],
                                    ets[i][:, u, r * P : (r + 1) * P],
                                    vaug_sb[i][:],
                                    start=(i == 0),
                                    stop=(i == t),
                                )
                    with nc.named_scope("norm"):
                        y2 = ypool.tile([P, P], bf, tag="y2", name="y2")
                        rc = ypool.tile([P, 2], f32, tag="rc", name="rc")
                        nc.vector.reciprocal(
                            rc[:], pav[:, HD : 2 * HD + 2 : HD + 1]
                        )
                        for u in range(2):
                            c0 = u * (HD + 1)
                            nc.vector.tensor_scalar(
                                y2[:, u * HD : (u + 1) * HD],
                                pav[:, c0 : c0 + HD],
                                rc[:, u : u + 1],
                                None,
                                mybir.AluOpType.mult,
                            )
                        ytmp = ypool.tile([P, P], bf, tag="ytmp", name="ytmp")
                        nc.sync.dma_start_transpose(ytmp[:], y2[:])
                        # y -> fp8 hi/lo on Pool (SBUF-only, legal there)
                        tb = slice(t * P, (t + 1) * P)
                        nc.gpsimd.tensor_copy(yt8hi_sb[:, m, tb], ytmp[:])
                        nc.gpsimd.tensor_sub(
                            yt8lo_sb[:, m, tb], ytmp[:], yt8hi_sb[:, m, tb]
                        )

            def wo_sm(sm, n):
                srow = slice(sm * P, (sm + 1) * P)
                for half in range(2):
                    ot = opool.tile([P, 2 * CH], bf, tag="ot", name="ot")
                    for q in range(2):
                        dcJ = 2 * half + q
                        dch = slice(dcJ * CH, (dcJ + 1) * CH)
                        pw = ps_a.tile([P, CH], f32, tag="proj", name="ps_wo")
                        with nc.named_scope("wo"):
                            for ti, (yw, ww) in enumerate(
                                (
                                    (yt8hi_sb, wothi_sb),
                                    (yt8lo_sb, wothi_sb),
                                    (yt8hi_sb, wotlo_sb),
                                )
                            ):
                                nc.tensor.matmul(
                                    pw[:],
                                    yw[:, :, srow],
                                    ww[:, :, dch],
                                    start=(ti == 0),
                                    stop=(ti == 2),
                                    perf_mode=DR,
                                )
                        with nc.named_scope("outdma"):
                            # GPSIMD cannot read PSUM: drains go DVE, with
                            # the post-exp last chunk split DVE/ACT
                            if sm >= 12 and (2 * n + q) % 2 == 0:
                                nc.scalar.copy(ot[:, q * CH : (q + 1) * CH], pw[:])
                            else:
                                nc.vector.tensor_copy(ot[:, q * CH : (q + 1) * CH], pw[:])
                    with nc.named_scope("outdma"):
                        h0 = half * 2 * CH
                        if sm == 15:
                            nc.sync.dma_start(out_d[srow, h0 : h0 + CH], ot[:, 0:CH])
                            nc.scalar.dma_start(
                                out_d[srow, h0 + CH : h0 + 2 * CH], ot[:, CH : 2 * CH]
                            )
                        else:
                            nc.sync.dma_start(out_d[srow, h0 : h0 + 2 * CH], ot[:])

            # ---- software-pipelined main loop ----------------------------
            # P(j) proj+rope | S(j) scores+exp | V(j) av+norm | W(j) wo.
            # Interleaved so PE streams P/W work while ACT runs exp ahead.
            def proj_chunk0(j):
                # early chunks are DMA-feed-bound: interleave all three
                # m-tiles kp-major so PE tracks the x/w arrival.
                js = slice(j * CH, (j + 1) * CH)
                psA = ps_a.tile([P, CH], f32, tag="proj", name="ps_pA")
                psB = ps_a.tile([P, CH], f32, tag="proj", name="ps_pB")
                psC3 = ps_s.tile([P, 2, CH], f32, tag="sc", name="ps_pC")
                psC = psC3[:, 0, :]
                pslist = (psA, psB, psC)
                with nc.named_scope("proj"):
                    for kp in range(NKP):
                        for mi, m in enumerate((2, 0, 1)):
                            for ti, (wt, xt) in enumerate(
                                (
                                    (whi_sb[kp], xhi_sb[kp]),
                                    (whi_sb[kp], xlo_sb[kp]),
                                    (wlo_sb[kp], xhi_sb[kp]),
                                )
                            ):
                                nc.tensor.matmul(
                                    pslist[mi],
                                    wt[:, :, m * P : (m + 1) * P],
                                    xt[:, :, js],
                                    start=(kp == 0 and ti == 0),
                                    stop=(kp == NKP - 1 and ti == 2),
                                    perf_mode=DR,
                                )
                return psA, psB, psC

            def st_p(j):
                if j <= 1:
                    psA, psB, psC = proj_chunk0(j)
                    rope_kv(psA, j)
                    rope128(qt_sb[0], psB, j)
                    rope128(qt_sb[1], psC, j)
                else:
                    rope_kv(proj_psum(2, j), j)
                    rope128(qt_sb[0], proj_psum(0, j), j)
                    rope128(qt_sb[1], proj_psum(1, j), j)
                vtrans(j)

            def st_s(j):
                scores_chunk(j, 0)
                scores_chunk(j, 1)

            def st_v(j):
                av_chunk(j, 0)
                av_chunk(j, 1)

            def st_w(j):
                for sm in range(4 * j, 4 * j + 4):
                    wo_sm(sm, sm)

            st_p(0)
            st_p(1)
            st_s(0)
            st_p(2)
            st_v(0)
            st_s(1)
            st_p(3)
            st_v(1)
            st_s(2)
            st_v(2)
            scores_chunk(3, 0, range(0, 8))
            wo_sm(0, 0)
            wo_sm(1, 1)
            scores_chunk(3, 0, range(8, 16))
            wo_sm(2, 2)
            wo_sm(3, 3)
            scores_chunk(3, 1, range(0, 8))
            wo_sm(4, 4)
            wo_sm(5, 5)
            scores_chunk(3, 1, range(8, 16))
            wo_sm(6, 6)
            wo_sm(7, 7)
            av_chunk(3, 0)
            st_w(2)
            for r in range(4):
                av_chunk(3, 1, (r,))
                wo_sm(12 + r, 12 + r)

    nc.finalize()
    return nc


def _host_inputs(x, freqs_cos, freqs_sin, wq, wk, wv, wo):
    """Build the 8 per-core input maps (all host-side preprocessing)."""
    x = np.asarray(x, np.float32)
    cos = np.asarray(freqs_cos, np.float32)  # [S, 32]
    sin = np.asarray(freqs_sin, np.float32)
    wq = np.asarray(wq, np.float32)
    wk = np.asarray(wk, np.float32)
    wv = np.asarray(wv, np.float32)
    wo = np.asarray(wo, np.float32)

    perm = np.concatenate([np.arange(0, HD, 2), np.arange(1, HD, 2)])  # de-interleave

    def dr_pairs(a):
        # [D, F] -> [8*128, 2*F] DoubleRow pair layout
        f = a.shape[1]
        return np.ascontiguousarray(
            a.reshape(NKP, 2, P, f).transpose(0, 2, 1, 3).reshape(NKP * P, 2 * f)
        )

    xt = np.asarray(x[0].T, dtype=BF16).astype(np.float32)  # [D, S]
    x_hi = xt.astype(F8)
    x_lo = (xt - x_hi.astype(np.float32)).astype(F8)
    xhi_p = dr_pairs(x_hi.astype(np.float32)).astype(F8)
    xlo_p = dr_pairs(x_lo.astype(np.float32)).astype(F8)

    # rope tables, stacked for two heads
    cos128 = np.empty((P, S), np.float16)
    swap128 = np.empty((P, S), np.float16)
    for dd in range(P):
        i = dd % 32
        r = dd % HD
        cos128[dd] = cos[:, i].astype(np.float16)
        swap128[dd] = (sin[:, i] if r < 32 else -sin[:, i]).astype(np.float16)

    pp = np.arange(P)[:, None]
    ff = np.arange(P)[None, :]
    masks2 = np.tile((pp <= ff).astype(np.float32), (1, 2)).astype(BF16)

    ident = np.eye(HD, dtype=np.float32).astype(BF16)

    in_maps = []
    for c in range(NCORES):
        wq_c = wq[c * QCOLS : (c + 1) * QCOLS].reshape(QH, HD, D)[:, perm, :].reshape(
            QCOLS, D
        )
        wk_c = wk[c * HD : (c + 1) * HD][perm, :]
        wv_c = wv[c * HD : (c + 1) * HD]
        wqkvt = np.concatenate([wq_c, wk_c, wv_c], axis=0).T * WS  # [D, 384]
        w_hi = wqkvt.astype(F8)
        w_lo = (wqkvt - w_hi.astype(np.float32)).astype(F8)
        whi_p = dr_pairs(w_hi.astype(np.float32)).astype(F8)
        wlo_p = dr_pairs(w_lo.astype(np.float32)).astype(F8)
        wot = np.ascontiguousarray(wo[:, c * QCOLS : (c + 1) * QCOLS].T) * WS
        wot_hi = wot.astype(F8)
        wot_lo = (wot - wot_hi.astype(np.float32)).astype(F8)

        def wot_pairs(a):
            # [256, D] -> [128, 2*D]: row p holds (row p | row 128+p)
            return np.ascontiguousarray(
                a.reshape(2, P, D).transpose(1, 0, 2).reshape(P, 2 * D)
            )

        wothi_p = wot_pairs(wot_hi.astype(np.float32)).astype(F8)
        wotlo_p = wot_pairs(wot_lo.astype(np.float32)).astype(F8)
        in_maps.append(
            {
                "xhi": xhi_p,
                "xlo": xlo_p,
                "whi": whi_p,
                "wlo": wlo_p,
                "wothi": wothi_p,
                "wotlo": wotlo_p,
                "cos128": cos128,
                "swap128": swap128,
                "masks2": masks2,
                "ident": ident,
            }
        )
    return in_maps


def kernel(x, freqs_cos, freqs_sin, wq, wk, wv, wo):
    from concourse.bass_utils import run_bass_kernel_spmd

    if "nc" not in _CACHE:
        _CACHE["nc"] = _build()
    nc = _CACHE["nc"]
    in_maps = _host_inputs(x, freqs_cos, freqs_sin, wq, wk, wv, wo)
    res = run_bass_kernel_spmd(nc, in_maps, core_ids=list(range(NCORES)))
    out = np.zeros((S, D), np.float64)
    for r in res.results:
        out += r["out"].astype(np.float64)
    return (out / WS).astype(np.float32).reshape(1, S, D)
